# revision 42
# baseline (speedup 1.0000x reference)
"""Bahdanau attention kernel for Trainium2 (8 NeuronCores, data-parallel over batch).

Reference computation (B=32, T=4096, D=U=512):
    q_proj = query @ W1 + b1                      [B, 1, U]
    v_proj = values @ W2 + b2                     [B, T, U]
    scores = tanh(q_proj + v_proj) @ V + bv       [B, T, 1]
    attn   = softmax(scores, axis=1)
    out    = sum(attn * values, axis=1)           [B, D]

Device strategy (per core, 4 batches), using only PE + ACT + DMA (the DVE is
~16x slower than spec on this runtime — measured 4.4us for a [128,512] copy —
and the accum-out paths are unusable):
  - Host folds b1/b2 into q_eff = query@W1 + b1 + b2, drops bv (softmax shift
    invariant), ships values twice: natural [T, D] bf16 (context matmul) and
    transposed [D, T] fp8 (projection matmul) so the device reads each
    element once and never transposes.
  - v_proj computed transposed [U, t] with W2 stationary (LDWEIGHTS amortizes
    over t), fp8 DoubleRowSwInterleave (2 matmuls of K=256, host-interleaved
    W2 pre-scaled by F8_SCALE, un-scaled inside the ACT tanh); q_eff rides
    the tanh per-partition bias for free.
  - scores: tanh tiles (fp8 => FWL weight loads) become the stationary
    operand against V [128,1]; scores land in [128, T/128] partition-major
    PSUM layout; ~25ns per LDW+1-col-MM pair through the PE reorder window.
  - softmax without division or max-subtraction: unnormalized exp on device,
    colsum via a ones-matmul, division on host.
  - Context: 4 t-blocks at a time as CONCURRENT col-group matmuls
    (tile_position=(0,32j), M=1 each); partial rows land on partitions
    0/32/64/96 of one PSUM bank and are summed on host.
  - "quad" mode runs a slot-based software pipeline (slot = one 1024-col
    t-pair of one batch): scores are delayed one slot so their tanh inputs
    are always done, ctx quads two slots and placed between ub-projections
    where proj would stall on the vp PSUM ring; vp is triple-buffered
    (scoresP+colsum share one PSUM bank, ctx accumulator uses one) which
    decouples the proj->tanh->proj round-trip that otherwise paces both
    engines at ~1.2us/ub. PE and ACT both land at ~85us busy of ~103us total.
  - All DMAs stay on the single in-order sync queue: the queue IS the
    back-pressure that paces vN loads against compute; separate queues
    front-load vN and congest HBM right when vT(b0/b1) must stream (measured
    6-14us vT transfer landings, PE starved). gpsimd-queue DMAs also land
    ~4us late (slow engine) — never put latency-critical loads there.
"""

import os
import sys

import numpy as np

try:
    import ml_dtypes  # noqa: F401
except ImportError:  # pragma: no cover
    sys.path.insert(0, "/opt/trn_rl_repo")
    import ml_dtypes  # noqa: F401

try:
    import concourse  # noqa: F401
except ImportError:  # pragma: no cover
    sys.path.insert(0, "/opt/trn_rl_repo")

BF16 = np.dtype(ml_dtypes.bfloat16)
FP8 = np.dtype(ml_dtypes.float8_e4m3)

B, T, D, U = 32, 4096, 512, 512
N_CORES = 8
BPC = B // N_CORES  # batches per core

F8_SCALE = 64.0  # host scales W2 by this; ACT tanh un-scales via scale=1/F8_SCALE

# fp8swi = fp8 DoubleRowSwInterleave (host-interleaved W2): contiguous
# LDWEIGHTS reads, measured faster than plain DoubleRow
# quad   = fp8swi + col-group-packed context matmuls + slot software pipeline
MODE = os.environ.get("BAHDANAU_MODE", "quad")  # "quad" | "fp8swi" | "fp8" | "bf16"

_MODULES: dict = {}


def _build(bpc: int = BPC, t: int = T, mode: str = "fp8"):
    """Build + compile the per-core Bass module. Shapes are per-core shards."""
    from contextlib import ExitStack

    import concourse.bass as bass
    import concourse.tile as tile
    from concourse import bacc, mybir

    f32 = mybir.dt.float32
    bf16 = mybir.dt.bfloat16
    fp8 = mybir.dt.float8e4
    FT = mybir.ActivationFunctionType
    ALU = mybir.AluOpType
    PSUM = bass.MemorySpace.PSUM
    use_quad = mode == "quad"
    use_swi = mode in ("fp8swi", "quad")
    DR = (
        mybir.MatmulPerfMode.DoubleRowSwInterleave
        if use_swi
        else mybir.MatmulPerfMode.DoubleRow
    )

    use_fp8 = mode in ("fp8", "fp8swi", "quad")
    vt_dt = fp8 if use_fp8 else bf16
    tb_n = t // 128  # 128-row t-blocks per batch
    tc_n = t // 512  # 512-col t-chunks per batch
    tanh_scale = (1.0 / F8_SCALE) if use_fp8 else 1.0

    nc = bacc.Bacc(
        "TRN2", target_bir_lowering=False, debug=False, enable_asserts=False
    )

    vT_d = nc.dram_tensor("valuesT", [bpc, D, t], vt_dt, kind="ExternalInput")
    vN_d = nc.dram_tensor("valuesN", [bpc, t, D], bf16, kind="ExternalInput")
    if use_swi:
        # pre-interleaved DoubleRowSwInterleave weight layout:
        # [p, j, ub, 2c+i] = W2[(2j+i)*128+p, ub*128+(127-c)]
        w2_d = nc.dram_tensor("w2t", [128, 2, 4, 256], vt_dt, kind="ExternalInput")
    else:
        w2_d = nc.dram_tensor("w2t", [D, U], vt_dt, kind="ExternalInput")
    # one packed bf16 tensor for all small consts: [0:4]=V columns,
    # [4:4+bpc*4]=q_eff (b-major, ub-minor), [-1]=ones column
    smalls_d = nc.dram_tensor(
        "smalls", [128, 4 + bpc * 4 + 1], bf16, kind="ExternalInput"
    )
    if use_quad:
        # context partials: 4 col-group rows (partitions 0/32/64/96) in ONE
        # psum bank per batch (bf16, summed on host) with the exp-colsums in
        # 32 extra columns of partition 0 — ONE output DMA per batch
        ctxp_d = nc.dram_tensor(
            "ctx_part", [bpc, 128, D + 32], bf16, kind="ExternalOutput"
        )
        out_d = None
    else:
        # per batch: two partial context rows (even/odd t-blocks, summed on
        # host) then the tb_n exp-colsums — one DMA per batch
        out_d = nc.dram_tensor(
            "ctx_out", [bpc, 2 * D + tb_n], f32, kind="ExternalOutput"
        )

    with tile.TileContext(nc) as tc, ExitStack() as ctx:
        const = ctx.enter_context(tc.tile_pool(name="const", bufs=1))
        vT_pool = ctx.enter_context(tc.tile_pool(name="vT", bufs=3))
        vN_pool = ctx.enter_context(tc.tile_pool(name="vN", bufs=3))
        tanh_pool = ctx.enter_context(tc.tile_pool(name="tanh", bufs=8))
        # one pool for all small SBUF tiles (expP, colsum row, ctx row):
        # fewer pools = fewer teardown semaphores in the Tile epilogue
        misc_pool = ctx.enter_context(tc.tile_pool(name="misc", bufs=2))
        sm_pool = misc_pool
        attn_pool = misc_pool
        ctxs_pool = misc_pool
        vp_psum = ctx.enter_context(
            tc.tile_pool(name="vp_ps", bufs=(3 if use_quad else 2), space=PSUM)
        )
        # bufs=1: batch b+1's scores start ~4us after exp(b) finished reading
        # scoresP, so single-buffering costs nothing and frees a bank for the
        # two-bank ctx accumulator
        sco_psum = ctx.enter_context(tc.tile_pool(name="sc_ps", bufs=1, space=PSUM))
        ctx_psum = ctx.enter_context(tc.tile_pool(name="ctx_ps", bufs=1, space=PSUM))
        sms_psum = ctx.enter_context(tc.tile_pool(name="sm_ps", bufs=1, space=PSUM))

        # ALL DMAs stay interleaved on the single sync queue: the in-order
        # queue is the back-pressure that paces vN loads to compute progress
        # — giving vN its own queue front-loads 12MB of vN and congests HBM
        # right when vT(b0/b1) must stream (measured: 6-14us vT DMAs, PE
        # starved). The gpsimd queue's DMAs also LAND slowly (w2 256KB took
        # 4.5us to raise its semaphore there, stalling the first matmul).
        dma_vT = nc.sync.dma_start
        dma_vN = nc.sync.dma_start
        dma_misc = nc.sync.dma_start

        if use_swi:
            w2_sb = const.tile([128, 2, 4, 256], vt_dt)
            dma_misc(w2_sb[:, 0], w2_d.ap()[:, 0])
            dma_misc(w2_sb[:, 1], w2_d.ap()[:, 1])
        else:
            w2_sb = const.tile([128, 4, U], vt_dt)
            dma_misc(w2_sb[:], w2_d.ap().rearrange("(db p) u -> p db u", p=128))
        smalls_sb = const.tile([128, 4 + bpc * 4 + 1], bf16)
        vc_sb = smalls_sb[:, 0:4]
        qe_sb = smalls_sb[:, 4 : 4 + bpc * 4].rearrange("p (b ub) -> p b ub", b=bpc)
        c1b_sb = smalls_sb[:, 4 + bpc * 4 : 4 + bpc * 4 + 1]

        vT_sbs: dict = {}

        def vT_fetch(b, slices):
            """Emit vT chunk DMAs for batch b (allocating its tile on first
            use). Called from the PREVIOUS batch's pair loop so vT(b+1)
            interleaves with vN(b) on the sync ring instead of queueing
            behind it."""
            if b >= bpc:
                return
            if b not in vT_sbs:
                vT_sbs[b] = vT_pool.tile([128, 4, t], vt_dt, name=f"vT{b}", tag="vT")
            src = vT_d[b].rearrange("(db p) tt -> p db tt", p=128)
            for sl in slices:
                dma_vT(vT_sbs[b][:, :, sl], src[:, :, sl])

        # first projection needs only w2 + vT chunk 0: get those onto the
        # sync ring before the smalls const (needed ~1us later for tanh)
        vT_fetch(0, [slice(0, 512), slice(512, 1024)])
        dma_misc(smalls_sb[:], smalls_d.ap())

        def stage(b, prev_tail):
            """Full per-batch pipeline: load, project, scores, exp, context.

            The previous batch's final-pair context/sum work (`prev_tail`) is
            emitted after this batch's first pair so the PE never stalls the
            ACT pipeline at batch boundaries. Returns this batch's tail."""
            n_pairs = tc_n // 2
            if b == 0:  # chunks 0-1 were fetched in the preamble
                if use_quad:
                    vT_fetch(0, [slice(c * 1024, (c + 1) * 1024) for c in range(1, t // 1024)])
                else:
                    vT_fetch(0, [slice(c * 512, (c + 1) * 512) for c in range(2, t // 512)])
            vT_sb = vT_sbs[b]
            vN_sb = vN_pool.tile([128, tb_n, D], bf16)
            vN_src = vN_d[b].rearrange("(n p) dd -> p n dd", p=128)
            scoresP = sco_psum.tile([128, tb_n], f32)
            expP = attn_pool.tile([128, tb_n], bf16, tag="exp")
            if use_quad:
                # 4 t-blocks per quad run CONCURRENTLY in the PE's 4
                # col-groups (tile_position): partial rows land on
                # partitions 0/32/64/96; 2 banks alternate per quad so
                # fill/drain overlap. Host sums the 8 partial rows.
                cps = ctx_psum.tile([128, 2, D], f32)
            else:
                # two banks, even/odd t-blocks: consecutive accumulating
                # matmuls alternate banks so fill/drain overlap (same-bank
                # accumulation serializes at ~321ns/MM vs ~213ns streaming)
                cps = ctx_psum.tile([1, 2, D], f32)

            n_quads = tb_n // 4

            def ctx_mms(pair, ks):
                if use_quad:
                    for q in ks:
                        g = pair * 2 + q  # global quad index
                        for j in range(4):
                            n = 4 * g + j
                            # NOTE: measured on HW — the start=True
                            # has_written clear is scoped to the partitions
                            # the tile writes (NOT the whole bank), so every
                            # col-group tile needs its own start
                            nc.tensor.matmul(
                                cps[32 * j : 32 * j + 1, g % 2, :],
                                expP[:, n : n + 1],
                                vN_sb[:, n, :],
                                start=(g < 2),
                                stop=(g >= n_quads - 2),
                                tile_position=(0, 32 * j),
                            )
                    return
                for k in ks:
                    n = pair * 8 + k
                    nc.tensor.matmul(
                        cps[:, n % 2, :],
                        expP[:, n : n + 1],
                        vN_sb[:, n, :],
                        start=(n < 2),
                        stop=(n >= tb_n - 2),
                    )

            for pair in range(n_pairs):
                # batch 0 loads vN per pair (latency); later batches use
                # 2-pair transfers — fewer DMAs = fewer epilogue semaphores
                if b == 0 and not use_quad:
                    n_sl = slice(pair * 8, (pair + 1) * 8)
                    dma_vN(vN_sb[:, n_sl, :], vN_src[:, n_sl, :])
                elif pair % 2 == 0:
                    n_sl = slice(pair * 8, (pair + 2) * 8)
                    dma_vN(vN_sb[:, n_sl, :], vN_src[:, n_sl, :])
                # prefetch the NEXT batch's vT (2-pair chunks)
                if pair % 2 == 0:
                    vT_fetch(b + 1, [slice(pair * 1024, (pair + 2) * 1024)])
                tanh_tiles = []
                for ub in range(4):
                    vp = vp_psum.tile([128, 2, 512], f32, tag="vp")
                    # j outer / half inner so consecutive matmuls share the
                    # same stationary W2 block (LDWEIGHTS amortization)
                    if use_fp8:
                        for j in range(2):
                            for half in range(2):
                                tc8 = pair * 2 + half
                                if use_swi:
                                    stat = w2_sb[:, j, ub, :]
                                else:
                                    stat = w2_sb[
                                        :, 2 * j : 2 * j + 2, bass.ts(ub, 128)
                                    ]
                                mov = vT_sb[:, 2 * j : 2 * j + 2, bass.ts(tc8, 512)]
                                nc.tensor.matmul(
                                    vp[:, half, :], stat, mov,
                                    start=(j == 0), stop=(j == 1),
                                    perf_mode=DR,
                                )
                    else:
                        for j in range(4):
                            for half in range(2):
                                tc8 = pair * 2 + half
                                nc.tensor.matmul(
                                    vp[:, half, :],
                                    w2_sb[:, j, bass.ts(ub, 128)],
                                    vT_sb[:, j, bass.ts(tc8, 512)],
                                    start=(j == 0),
                                    stop=(j == 3),
                                )
                    # fp8 tanh output => scores LDWEIGHTS gets FWL (4x faster
                    # weight load); |tanh|<=1 so e4m3 is safe
                    th = tanh_pool.tile([128, 2, 512], fp8 if use_fp8 else bf16)
                    nc.scalar.activation(
                        th[:],
                        vp[:],
                        FT.Tanh,
                        bias=qe_sb[:, b, ub : ub + 1],
                        scale=tanh_scale,
                    )
                    tanh_tiles.append(th)
                # first half of the previous pair's context matmuls BEFORE
                # this pair's scores (covers ACT finishing the tanh tiles);
                # second half AFTER scores (covers the tanh that frees the
                # vp slot for the next pair's projection)
                if pair == 0 and prev_tail is not None:
                    prev_tail()
                if pair > 0:
                    ctx_mms(pair - 1, [0] if use_quad else range(4))
                # NOTE: ub must be the inner loop — start=True clears
                # has_written for the WHOLE psum bank, so each column's
                # 4-matmul accumulation group must complete before the next
                # column's group starts
                for tl8 in range(8):
                    blk = pair * 8 + tl8
                    for ub in range(4):
                        nc.tensor.matmul(
                            scoresP[:, blk : blk + 1],
                            tanh_tiles[ub][:, tl8 // 4, bass.ts(tl8 % 4, 128)],
                            vc_sb[:, ub : ub + 1],
                            start=(ub == 0),
                            stop=(ub == 3),
                        )
                if pair > 0:
                    ctx_mms(pair - 1, [1] if use_quad else range(4, 8))
                # unnormalized softmax weights for this pair; division by the
                # sum of exp happens on the host via the colsums output
                nc.scalar.activation(
                    expP[:, pair * 8 : (pair + 1) * 8],
                    scoresP[:, pair * 8 : (pair + 1) * 8],
                    FT.Exp,
                )

            def tail():
                ctx_mms(n_pairs - 1, [0, 1] if use_quad else range(8))
                # colsums as a [1, tb_n] row (ones stationary, expP moving) so
                # everything lands on partition 0 and ships in ONE output DMA
                ps1 = sms_psum.tile([1, tb_n], f32, tag="sm")
                nc.tensor.matmul(ps1[:], c1b_sb, expP[:], start=True, stop=True)
                if use_quad:
                    csb = ctxs_pool.tile([128, 2, D + 16], bf16, tag="cs")
                    nc.scalar.copy(csb[:, :, 0:D], cps[:])
                    nc.scalar.copy(
                        csb[0:1, :, D : D + 16],
                        ps1[:].rearrange("a (i j) -> a i j", i=2),
                    )
                    dma_misc(ctxp_d[b], csb[:])
                    return
                cs_raw = ctxs_pool.tile([1, 2 * D + tb_n], f32, tag="cs")
                nc.scalar.copy(cs_raw[:, 0 : 2 * D], cps[:])
                nc.scalar.copy(cs_raw[:, 2 * D :], ps1[:])
                # NOTE: DMAs stay on the sync ring — a DMA on the scalar/ACT
                # ring occupies that queue for its full duration and stalls
                # the next batch's tanh pipeline
                nc.sync.dma_start(out_d[b : b + 1, :], cs_raw[:])

            return tail

        def quad_pipeline():
            """Slot-based deep software pipeline (quad mode).

            Slot P = (batch b, pair p).  scores are DELAYED one slot (so the
            tanh tiles they consume are guaranteed done — no PE wait on ACT)
            and ctx quads two slots; the ctx quads are placed between the ub
            projections exactly where proj would otherwise stall on the vp
            ring (tanh per-ub ~1013ns > proj per-ub ~860ns); vp bufs=3
            decouples the proj->tanh->proj round-trip latency.

            Per-slot emission order:
              proj ub0, proj ub1, ctx(P-2,q0), proj ub2, ctx(P-2,q1),
              proj ub3, scores(P-1), exp(P-1)[, colsum][, tailcopy+DMA]
            """
            n_pairs = tc_n // 2
            n_slots = bpc * n_pairs
            n_quads = tb_n // 4
            vN_sbs: dict = {}
            scoresPs: dict = {}
            expPs: dict = {}
            cpss: dict = {}
            ps1s: dict = {}
            tanh_slots: dict = {}

            def ensure_batch(b):
                if b in vN_sbs:
                    return
                vN_sbs[b] = vN_pool.tile([128, tb_n, D], bf16, name=f"vN{b}", tag="vN")
                # scoresP (cols 0:32) and the colsum row (partition 0, cols
                # 32:64) share one PSUM bank — frees banks for vp bufs=3
                scob = sco_psum.tile([128, 2 * tb_n], f32, name=f"scob{b}", tag="sco")
                scoresPs[b] = scob[:, 0:tb_n]
                ps1s[b] = scob[0:1, tb_n : tb_n + tb_n]
                expPs[b] = attn_pool.tile([128, tb_n], bf16, name=f"expP{b}", tag="exp")
                cpss[b] = ctx_psum.tile([128, D], f32, name=f"cps{b}", tag="cps")

            def proj_ub(b, pair, ub):
                vp = vp_psum.tile([128, 2, 512], f32, tag="vp")
                for j in range(2):
                    for half in range(2):
                        tc8 = pair * 2 + half
                        stat = w2_sb[:, j, ub, :]
                        mov = vT_sbs[b][:, 2 * j : 2 * j + 2, bass.ts(tc8, 512)]
                        nc.tensor.matmul(
                            vp[:, half, :], stat, mov,
                            start=(j == 0), stop=(j == 1),
                            perf_mode=DR,
                        )
                th = tanh_pool.tile([128, 2, 512], fp8)
                nc.scalar.activation(
                    th[:], vp[:], FT.Tanh,
                    bias=qe_sb[:, b, ub : ub + 1], scale=tanh_scale,
                )
                return th

            def ctx_quad(S, q):
                b, pair = divmod(S, n_pairs)
                g = pair * 2 + q
                for j in range(4):
                    n = 4 * g + j
                    # NOTE: measured on HW — the start=True has_written clear
                    # is scoped to the partitions the tile writes (NOT the
                    # whole bank), so every col-group tile needs its own start
                    nc.tensor.matmul(
                        cpss[b][32 * j : 32 * j + 1, :],
                        expPs[b][:, n : n + 1],
                        vN_sbs[b][:, n, :],
                        start=(g == 0),
                        stop=(g == n_quads - 1),
                        tile_position=(0, 32 * j),
                    )

            def scores_grp(S, tiles, grp):
                """Two score blocks (8 matmuls) of slot S — emitted as filler
                between the next slot's ub projections so the in-order PE
                queue has ready work wherever proj would stall on a vp slot.

                NOTE: ub must be the inner loop — start=True clears
                has_written for the bank columns of the written partitions,
                so each column's 4-matmul accumulation group must complete
                before the next column's group starts"""
                b, pair = divmod(S, n_pairs)
                for tl8 in (2 * grp, 2 * grp + 1):
                    blk = pair * 8 + tl8
                    for ub in range(4):
                        nc.tensor.matmul(
                            scoresPs[b][:, blk : blk + 1],
                            tiles[ub][:, tl8 // 4, bass.ts(tl8 % 4, 128)],
                            vc_sb[:, ub : ub + 1],
                            start=(ub == 0),
                            stop=(ub == 3),
                        )

            def exp_emit(S):
                b, pair = divmod(S, n_pairs)
                nc.scalar.activation(
                    expPs[b][:, pair * 8 : (pair + 1) * 8],
                    scoresPs[b][:, pair * 8 : (pair + 1) * 8],
                    FT.Exp,
                )

            csbs: dict = {}

            def colsum(b):
                nc.tensor.matmul(ps1s[b][:], c1b_sb, expPs[b][:], start=True, stop=True)
                # copy the colsum row out right away: batch b+1's first scores
                # matmul reuses this PSUM bank (WAR) and must not wait long
                csbs[b] = ctxs_pool.tile([128, D + 32], bf16, name=f"csb{b}", tag="cs")
                nc.scalar.copy(csbs[b][0:1, D : D + 32], ps1s[b][:])

            def tailcopy(b):
                csb = csbs.pop(b)
                nc.scalar.copy(csb[:, 0:D], cpss[b][:])
                dma_misc(ctxp_d[b], csb[:])

            for P in range(n_slots + 2):
                if P < n_slots:
                    b, pair = divmod(P, n_pairs)
                    ensure_batch(b)
                    if b == 0 and pair == 0:
                        vT_fetch(0, [slice(c * 1024, (c + 1) * 1024) for c in range(1, t // 1024)])
                    if pair % 2 == 0:
                        n_sl = slice(pair * 8, (pair + 2) * 8)
                        vN_src = vN_d[b].rearrange("(n p) dd -> p n dd", p=128)
                        dma_vN(vN_sbs[b][:, n_sl, :], vN_src[:, n_sl, :])
                        vT_fetch(b + 1, [slice(pair * 1024, (pair + 2) * 1024)])
                    # contiguous 16-MM proj stream (vp bufs=3 absorbs the
                    # tanh round-trip); ctx quads follow the scores run
                    tiles = [proj_ub(b, pair, ub) for ub in range(4)]
                    tanh_slots[P] = tiles
                prev = tanh_slots.pop(P - 1, None) if P >= 1 else None
                if prev is not None:
                    for grp in range(4):
                        scores_grp(P - 1, prev, grp)
                    exp_emit(P - 1)
                if P - 2 >= 0:
                    ctx_quad(P - 2, 0)
                    ctx_quad(P - 2, 1)
                if prev is not None:
                    if (P - 1) % n_pairs == n_pairs - 1:
                        colsum((P - 1) // n_pairs)
                if P >= 2 and (P - 2) % n_pairs == n_pairs - 1:
                    tailcopy((P - 2) // n_pairs)

        if use_quad:
            quad_pipeline()
        else:
            pend = None
            for b in range(bpc):
                pend = stage(b, pend)
            pend()

    nc.compile()
    return nc


def _get_module(bpc: int = BPC, t: int = T, mode: str | None = None):
    mode = MODE if mode is None else mode
    key = (mode, bpc, t)
    if key not in _MODULES:
        _MODULES[key] = _build(bpc, t, mode)
    return _MODULES[key]


def _prep_inputs(query, values, W1, b1, W2, b2, V, bv, mode: str | None = None):
    """Host-side preprocessing: fold biases, cast, transpose, shard."""
    mode = MODE if mode is None else mode
    query = np.asarray(query, np.float32)
    values = np.asarray(values, np.float32)
    W1 = np.asarray(W1, np.float32)
    b1 = np.asarray(b1, np.float32)
    W2 = np.asarray(W2, np.float32)
    b2 = np.asarray(b2, np.float32)
    V = np.asarray(V, np.float32)

    q_eff = (
        query.astype(np.float64) @ W1.astype(np.float64)
        + b1.astype(np.float64)
        + b2.astype(np.float64)
    ).astype(np.float32)  # [B, U]; bv dropped (softmax shift invariance)

    vN = values.astype(BF16)  # [B, T, D]
    vTf = np.ascontiguousarray(values.transpose(0, 2, 1))  # [B, D, T] fp32
    if mode in ("fp8", "fp8swi", "quad"):
        vT = vTf.astype(FP8)
        w2s = W2 * F8_SCALE
        if mode in ("fp8swi", "quad"):
            # [p, j, ub, 2c+i] = w2s[(2j+i)*128+p, ub*128+(127-c)]
            w4 = w2s.reshape(4, 128, 4, 128)  # [db, p, ub, u_in]
            w4 = w4[:, :, :, ::-1]  # reverse columns
            w4 = w4.reshape(2, 2, 128, 4, 128)  # [j, i, p, ub, c]
            w2 = np.ascontiguousarray(
                w4.transpose(2, 0, 3, 4, 1).reshape(128, 2, 4, 256)
            ).astype(FP8)  # [p, j, ub, (c i)]
        else:
            w2 = w2s.astype(FP8)
    else:
        vT = vTf.astype(BF16)
        w2 = W2.astype(BF16)
    in_maps = []
    for c in range(N_CORES):
        s = slice(c * BPC, (c + 1) * BPC)
        # packed small consts [128, 4 + BPC*4 + 1] bf16:
        # cols 0:4 = V (ub-major), 4:4+BPC*4 = q_eff (b-major, ub-minor), -1 = ones
        smalls = np.empty((128, 4 + BPC * 4 + 1), np.float32)
        smalls[:, 0:4] = V.reshape(4, 128).T  # smalls[p, ub] = V[ub*128+p]
        qe_c = q_eff[s].reshape(BPC, 4, 128)  # [b, ub, p]
        smalls[:, 4 : 4 + BPC * 4] = qe_c.transpose(2, 0, 1).reshape(128, BPC * 4)
        smalls[:, -1] = 1.0
        in_maps.append(
            {
                "valuesT": vT[s],
                "valuesN": vN[s],
                "w2t": w2,
                "smalls": smalls.astype(BF16),
            }
        )
    return in_maps


def _run(in_maps, trace=False, mode: str | None = None, **kw):
    from concourse.bass_utils import run_bass_kernel_spmd

    mode_r = MODE if mode is None else mode
    nc = _get_module(mode=mode)
    res = run_bass_kernel_spmd(
        nc, in_maps, core_ids=list(range(N_CORES)), trace=trace, **kw
    )
    if mode_r == "quad":
        ctxp = np.concatenate(
            [np.asarray(res.results[c]["ctx_part"]) for c in range(N_CORES)],
            axis=0,
        ).astype(np.float32)  # [B, 128, D+32]
        # partial context rows live on partitions 0/32/64/96; cols D:D+32 of
        # partition 0 hold the exp-colsums
        raw = ctxp[:, ::32, 0:D].sum(axis=1)  # [B, D]
        sums = ctxp[:, 0, D:]
        out = raw / sums.sum(axis=1, keepdims=True)
        return out, res
    packed = np.concatenate(
        [np.asarray(res.results[c]["ctx_out"]) for c in range(N_CORES)], axis=0
    ).astype(np.float32)  # [B, 2*D + T/128]
    raw = packed[:, :D] + packed[:, D : 2 * D]  # even/odd t-block partials
    sums = packed[:, 2 * D :]
    out = raw / sums.sum(axis=1, keepdims=True)
    return out, res


def kernel(query, values, W1, b1, W2, b2, V, bv):
    in_maps = _prep_inputs(query, values, W1, b1, W2, b2, V, bv)
    out, _ = _run(in_maps, trace=False)
    return out

